# revision 4
# baseline (speedup 1.0000x reference)
"""Trainium2 Bass kernel for GroupLinear:
    out = einsum('lgi,lgj,ogij->lo', x1, x2, W.reshape(O,g,b,b)) + bias

Equivalent to Y = outer @ W.T + b where outer[l, k] (k = g*b*b + i*b + j) is
the blockwise outer product x1[l,g,i]*x2[l,g,j] -- a [2048, 65536] @
[65536, 1024] matmul whose LHS is generated on the fly.

Sharding: tensor-parallel over the contraction dim. Core c owns weight
blocks g in {2c, 2c+1} (K_local = 8192), computes a full [2048, 1024] fp32
partial, and the host sums the 8 partials (+ bias). The replicated-x1
operand layout is prepared host-side (a pure gather -- no FLOPs); the
outer products themselves and all matmul FLOPs run on device (DVE + PE).

Device loop (per core, per l-block of 128 tokens):
  - DMA in x1rep slab [128, 8192] bf16 (partition = k%128, free = chunk*128+l)
  - DVE: 64x tensor_mul in-place -> T chunks [128, 128] (outer products)
  - PE : 2x64 accumulating matmuls T_c.T @ Wp_c -> psum [128 l, 512 o]
  - ACT: psum -> sbuf fp32, DMA out
Weights stay SBUF-resident ([128, 65536] bf16 = 131KB/partition).
"""

import sys
import os
import numpy as np

sys.path.insert(0, "/opt/trn_rl_repo")

import ml_dtypes  # noqa: E402

BF16 = ml_dtypes.bfloat16

L = 2048
H = 1024
O = 1024
B = 64
G = 16
NCORES = 8
GPC = G // NCORES          # weight blocks per core = 2
KL = GPC * B * B           # local contraction dim = 8192
NCHUNK = KL // 128         # 64 k-chunks of 128
LB = 128                   # l-block (tokens per psum tile)
NLB = L // LB              # 16
MM_N = int(os.environ.get("GL_MM_N", "512"))  # matmul moving free dim (ISA: psum bank caps fp32 out at 512)

_cache = {}


def _build_nc():
    from concourse import bass, tile, bacc

    mybir = bass.mybir
    bf = mybir.dt.bfloat16
    f32 = mybir.dt.float32

    nc = bacc.Bacc("TRN2", target_bir_lowering=False, debug=False)
    wp = nc.dram_tensor("wp", [128, NCHUNK * O], bf, kind="ExternalInput")
    x1r = nc.dram_tensor("x1r", [NLB, 128, KL], bf, kind="ExternalInput")
    x2s = nc.dram_tensor("x2s", [GPC, 128, L], bf, kind="ExternalInput")
    out = nc.dram_tensor("out", [L, O], f32, kind="ExternalOutput")

    with tile.TileContext(nc) as tc:
        with (
            tc.tile_pool(name="wpool", bufs=1) as wpool,
            tc.tile_pool(name="x2pool", bufs=1) as x2pool,
            tc.tile_pool(name="xpool", bufs=2) as xpool,
            tc.tile_pool(name="opool", bufs=2) as opool,
            tc.tile_pool(name="psum", bufs=2, space="PSUM") as psum,
        ):
            # inputs needed by the very first matmuls go first: x2 stacks,
            # the first x1rep slab, then the weight slab stream (fine-grained
            # so PE can start as soon as the first chunks land)
            x2t = []
            for g in range(GPC):
                t = x2pool.tile([128, L], bf, tag=f"x2_{g}")
                nc.sync.dma_start(t[:], x2s[g])
                x2t.append(t)
            xt0 = xpool.tile([128, KL], bf, tag="xt")
            nc.sync.dma_start(xt0[:], x1r[0])
            wt = wpool.tile([128, NCHUNK * O], bf)
            for s in range(16):
                sl = slice(s * (NCHUNK * O // 16), (s + 1) * (NCHUNK * O // 16))
                nc.sync.dma_start(wt[:, sl], wp[:, sl])

            for lb in range(NLB):
                if lb == 0:
                    xt = xt0
                else:
                    xt = xpool.tile([128, KL], bf, tag="xt")
                    nc.sync.dma_start(xt[:], x1r[lb])
                lsl = slice(lb * LB, (lb + 1) * LB)
                # outer products, in place: T_c = x1rep_c * x2stack
                for c in range(NCHUNK):
                    g = c >> 5
                    csl = slice(c * 128, (c + 1) * 128)
                    nc.vector.tensor_mul(xt[:, csl], xt[:, csl], x2t[g][:, lsl])
                if MM_N == 1024:
                    ps = psum.tile([128, O], f32)
                    for c in range(NCHUNK):
                        nc.tensor.matmul(
                            ps[:],
                            xt[:, c * 128:(c + 1) * 128],
                            wt[:, c * O:(c + 1) * O],
                            start=(c == 0),
                            stop=(c == NCHUNK - 1),
                        )
                    ot = opool.tile([128, O], f32)
                    nc.scalar.mul(ot[:], ps[:], 1.0)
                else:
                    ps0 = psum.tile([128, 512], f32)
                    ps1 = psum.tile([128, 512], f32)
                    for c in range(NCHUNK):
                        nc.tensor.matmul(
                            ps0[:],
                            xt[:, c * 128:(c + 1) * 128],
                            wt[:, c * O:c * O + 512],
                            start=(c == 0),
                            stop=(c == NCHUNK - 1),
                        )
                    for c in range(NCHUNK):
                        nc.tensor.matmul(
                            ps1[:],
                            xt[:, c * 128:(c + 1) * 128],
                            wt[:, c * O + 512:(c + 1) * O],
                            start=(c == 0),
                            stop=(c == NCHUNK - 1),
                        )
                    ot = opool.tile([128, O], f32)
                    nc.scalar.mul(ot[:, 0:512], ps0[:], 1.0)
                    nc.scalar.mul(ot[:, 512:O], ps1[:], 1.0)
                nc.sync.dma_start(out[lsl, :], ot[:])

    nc.compile()
    return nc


def _prep_inputs(input1, input2, W):
    """Host-side shard + layout (transposes / gathers / dtype casts only)."""
    x1 = np.ascontiguousarray(input1, dtype=np.float32)
    x2 = np.ascontiguousarray(input2, dtype=np.float32)
    Wt = np.ascontiguousarray(W.T, dtype=np.float32)  # [65536, 1024], k-major

    in_maps = []
    for core in range(NCORES):
        ks = slice(core * KL, (core + 1) * KL)
        gs = slice(core * GPC, (core + 1) * GPC)
        # weights: [k_local, o] -> [c, p, o] -> [p, c*O + o]
        wp = (
            Wt[ks]
            .reshape(NCHUNK, 128, O)
            .transpose(1, 0, 2)
            .reshape(128, NCHUNK * O)
            .astype(BF16)
        )
        # x1 replicated over j: k_local = g*B*B + i*B + j -> x1[l, g, i]
        x1g = x1.reshape(L, G, B)[:, gs, :].transpose(1, 2, 0)  # [g, i, l]
        rep = np.repeat(x1g, B, axis=1).reshape(KL, L)          # [k_local, l]
        x1r = (
            rep.reshape(NCHUNK, 128, NLB, LB)
            .transpose(2, 1, 0, 3)
            .reshape(NLB, 128, KL)
            .astype(BF16)
        )
        # x2 stacked twice along partitions: row p -> j = p % 64
        x2g = x2.reshape(L, G, B)[:, gs, :].transpose(1, 2, 0)  # [g, j, l]
        x2st = np.concatenate([x2g, x2g], axis=1).astype(BF16)  # [g, 128, l]
        in_maps.append(
            {
                "wp": np.ascontiguousarray(wp),
                "x1r": np.ascontiguousarray(x1r),
                "x2s": np.ascontiguousarray(x2st),
            }
        )
    return in_maps


def run(input1, input2, W, b, trace=False):
    """Shard, run on 8 NeuronCores, unshard. Returns (out, BassKernelResults)."""
    from concourse.bass_utils import run_bass_kernel_spmd

    if "nc" not in _cache:
        _cache["nc"] = _build_nc()
    nc = _cache["nc"]

    in_maps = _prep_inputs(input1, input2, W)
    res = run_bass_kernel_spmd(
        nc, in_maps, list(range(NCORES)), trace=trace
    )
    acc = np.zeros((L, O), dtype=np.float32)
    for core in range(NCORES):
        acc += res.results[core]["out"]
    acc += np.asarray(b, dtype=np.float32)[None, :]
    return acc, res


def kernel(input1, input2, W, b):
    out, _ = run(input1, input2, W, b, trace=False)
    return out


if __name__ == "__main__":
    rng = np.random.default_rng(0)
    x1 = rng.standard_normal((L, H), dtype=np.float32)
    x2 = rng.standard_normal((L, H), dtype=np.float32)
    W = rng.standard_normal((O, H * B), dtype=np.float32) / 256.0
    b = rng.standard_normal((O,), dtype=np.float32) / 256.0
    out = kernel(x1, x2, W, b)
    print("out", out.shape, out.dtype, float(np.abs(out).max()))


# revision 6
# speedup vs baseline: 1.6025x; 1.6025x over previous
"""Trainium2 Bass kernel for GroupLinear:
    out = einsum('lgi,lgj,ogij->lo', x1, x2, W.reshape(O,g,b,b)) + bias

Equivalent to Y = outer @ W.T + b where outer[l, k] (k = g*b*b + i*b + j) is
the blockwise outer product x1[l,g,i]*x2[l,g,j] -- a [2048, 65536] @
[65536, 1024] matmul whose LHS is generated on the fly.

Sharding: tensor-parallel over the contraction dim. Core c owns weight
blocks g in {2c, 2c+1} (K_local = 8192), computes a full [2048, 1024] fp32
partial, and the host sums the 8 partials (+ bias). The replicated-x1
operand layout is prepared host-side (a pure gather -- no FLOPs); the
outer products themselves and all matmul FLOPs run on device (DVE + PE).

Device loop (per core, per l-block of 128 tokens):
  - DMA in x1rep slab [128, 8192] bf16 (partition = k%128, free = chunk*128+l)
  - DVE: 64x tensor_mul in-place -> T chunks [128, 128] (outer products)
  - PE : 2x64 accumulating matmuls T_c.T @ Wp_c -> psum [128 l, 512 o]
  - ACT: psum -> sbuf fp32, DMA out
Weights stay SBUF-resident ([128, 65536] bf16 = 131KB/partition).
"""

import sys
import os
import numpy as np

sys.path.insert(0, "/opt/trn_rl_repo")

import ml_dtypes  # noqa: E402

BF16 = ml_dtypes.bfloat16

L = 2048
H = 1024
O = 1024
B = 64
G = 16
NCORES = 8
GPC = G // NCORES          # weight blocks per core = 2
KL = GPC * B * B           # local contraction dim = 8192
NCHUNK = KL // 128         # 64 k-chunks of 128
LB = 128                   # l-block (tokens per psum tile)
NLB = L // LB              # 16
MM_N = int(os.environ.get("GL_MM_N", "512"))  # matmul moving free dim (ISA: psum bank caps fp32 out at 512)

_cache = {}


def _build_nc(repeat=1):
    from concourse import bass, tile, bacc
    from contextlib import nullcontext

    mybir = bass.mybir
    bf = mybir.dt.bfloat16
    f32 = mybir.dt.float32

    nc = bacc.Bacc("TRN2", target_bir_lowering=False, debug=False)
    wp = nc.dram_tensor("wp", [128, NCHUNK * O], bf, kind="ExternalInput")
    x1r = nc.dram_tensor("x1r", [NLB, 128, KL], bf, kind="ExternalInput")
    x2s = nc.dram_tensor("x2s", [GPC, 128, L], bf, kind="ExternalInput")
    out = nc.dram_tensor("out", [L, O], f32, kind="ExternalOutput")

    with tile.TileContext(nc) as tc:
        with (
            tc.tile_pool(name="wpool", bufs=1) as wpool,
            tc.tile_pool(name="x2pool", bufs=1) as x2pool,
            tc.tile_pool(name="xpool", bufs=2) as xpool,
            tc.tile_pool(name="opool", bufs=2) as opool,
            tc.tile_pool(name="psum", bufs=2, space="PSUM") as psum,
            tc.For_i(0, repeat, 1) if repeat > 1 else nullcontext(),
        ):
            # inputs needed by the very first matmuls go first: x2 stacks,
            # the first x1rep slab, then the weight slab stream (fine-grained
            # so PE can start as soon as the first chunks land)
            x2t = []
            for g in range(GPC):
                t = x2pool.tile([128, L], bf, tag=f"x2_{g}")
                nc.sync.dma_start(t[:], x2s[g])
                x2t.append(t)
            xt0 = xpool.tile([128, KL], bf, tag="xt")
            nc.sync.dma_start(xt0[:], x1r[0])
            wt = wpool.tile([128, NCHUNK * O], bf)
            for s in range(16):
                sl = slice(s * (NCHUNK * O // 16), (s + 1) * (NCHUNK * O // 16))
                nc.sync.dma_start(wt[:, sl], wp[:, sl])

            for lb in range(NLB):
                if lb == 0:
                    xt = xt0
                else:
                    xt = xpool.tile([128, KL], bf, tag="xt")
                    nc.sync.dma_start(xt[:], x1r[lb])
                lsl = slice(lb * LB, (lb + 1) * LB)
                # outer products, in place: T_c = x1rep_c * x2stack
                for c in range(NCHUNK):
                    g = c >> 5
                    csl = slice(c * 128, (c + 1) * 128)
                    nc.vector.tensor_mul(xt[:, csl], xt[:, csl], x2t[g][:, lsl])
                if MM_N == 1024:
                    ps = psum.tile([128, O], f32)
                    for c in range(NCHUNK):
                        nc.tensor.matmul(
                            ps[:],
                            xt[:, c * 128:(c + 1) * 128],
                            wt[:, c * O:(c + 1) * O],
                            start=(c == 0),
                            stop=(c == NCHUNK - 1),
                        )
                    ot = opool.tile([128, O], f32)
                    nc.scalar.mul(ot[:], ps[:], 1.0)
                else:
                    ps0 = psum.tile([128, 512], f32)
                    ps1 = psum.tile([128, 512], f32)
                    for c in range(NCHUNK):
                        nc.tensor.matmul(
                            ps0[:],
                            xt[:, c * 128:(c + 1) * 128],
                            wt[:, c * O:c * O + 512],
                            start=(c == 0),
                            stop=(c == NCHUNK - 1),
                        )
                    for c in range(NCHUNK):
                        nc.tensor.matmul(
                            ps1[:],
                            xt[:, c * 128:(c + 1) * 128],
                            wt[:, c * O + 512:(c + 1) * O],
                            start=(c == 0),
                            stop=(c == NCHUNK - 1),
                        )
                    ot = opool.tile([128, O], f32)
                    nc.scalar.mul(ot[:, 0:512], ps0[:], 1.0)
                    nc.scalar.mul(ot[:, 512:O], ps1[:], 1.0)
                nc.sync.dma_start(out[lsl, :], ot[:])

    nc.compile()
    return nc


def _prep_inputs(input1, input2, W):
    """Host-side shard + layout (transposes / gathers / dtype casts only)."""
    x1 = np.ascontiguousarray(input1, dtype=np.float32)
    x2 = np.ascontiguousarray(input2, dtype=np.float32)
    Wt = np.ascontiguousarray(W.T, dtype=np.float32)  # [65536, 1024], k-major

    in_maps = []
    for core in range(NCORES):
        ks = slice(core * KL, (core + 1) * KL)
        gs = slice(core * GPC, (core + 1) * GPC)
        # weights: [k_local, o] -> [c, p, o] -> [p, c*O + o]
        wp = (
            Wt[ks]
            .reshape(NCHUNK, 128, O)
            .transpose(1, 0, 2)
            .reshape(128, NCHUNK * O)
            .astype(BF16)
        )
        # x1 replicated over j: k_local = g*B*B + i*B + j -> x1[l, g, i]
        x1g = x1.reshape(L, G, B)[:, gs, :].transpose(1, 2, 0)  # [g, i, l]
        rep = np.repeat(x1g, B, axis=1).reshape(KL, L)          # [k_local, l]
        x1r = (
            rep.reshape(NCHUNK, 128, NLB, LB)
            .transpose(2, 1, 0, 3)
            .reshape(NLB, 128, KL)
            .astype(BF16)
        )
        # x2 stacked twice along partitions: row p -> j = p % 64
        x2g = x2.reshape(L, G, B)[:, gs, :].transpose(1, 2, 0)  # [g, j, l]
        x2st = np.concatenate([x2g, x2g], axis=1).astype(BF16)  # [g, 128, l]
        in_maps.append(
            {
                "wp": np.ascontiguousarray(wp),
                "x1r": np.ascontiguousarray(x1r),
                "x2s": np.ascontiguousarray(x2st),
            }
        )
    return in_maps


def run(input1, input2, W, b, trace=False):
    """Shard, run on 8 NeuronCores, unshard. Returns (out, BassKernelResults)."""
    from concourse.bass_utils import run_bass_kernel_spmd

    if "nc" not in _cache:
        _cache["nc"] = _build_nc()
    nc = _cache["nc"]

    in_maps = _prep_inputs(input1, input2, W)
    res = run_bass_kernel_spmd(
        nc, in_maps, list(range(NCORES)), trace=trace
    )
    acc = np.zeros((L, O), dtype=np.float32)
    for core in range(NCORES):
        acc += res.results[core]["out"]
    acc += np.asarray(b, dtype=np.float32)[None, :]
    return acc, res


def kernel(input1, input2, W, b):
    out, _ = run(input1, input2, W, b, trace=False)
    return out


if __name__ == "__main__":
    rng = np.random.default_rng(0)
    x1 = rng.standard_normal((L, H), dtype=np.float32)
    x2 = rng.standard_normal((L, H), dtype=np.float32)
    W = rng.standard_normal((O, H * B), dtype=np.float32) / 256.0
    b = rng.standard_normal((O,), dtype=np.float32) / 256.0
    out = kernel(x1, x2, W, b)
    print("out", out.shape, out.dtype, float(np.abs(out).max()))
